# revision 9
# baseline (speedup 1.0000x reference)
"""Trainium2 Bass kernel for nn_Discriminator_712964571446.

Reference computation (N=512, n_B=32, n_C=16, CH=20):
    h  = relu(relu(x@W1+b1)@W2+b2)                 # (N, 20)
    M  = (h @ T).reshape(N, 32, 16)
    d[i,b,j] = sum_c |M[i,b,c] - M[j,b,c]|
    o  = sum_j exp(-d) ; o -= mean(o)
    out = sigmoid(concat([h, o]) @ W3 + b3)        # (N, 1)

Strategy (8 cores, data parallel over N, 64 rows per core):
  - tiny FC layers + full M^T recomputed on every core (cheaper than collectives)
  - M^T stored as 4 SBUF tiles of (128 bc-pairs, 512 j)
  - |x| = 2 relu(x) - x, so with S[b,j] = sum_c M[j,b,c]:
        d[b,j] = 2 * sum_c relu(M_j - M_i) - S[b,j] + S[b,i]
    per own row i: 4 fused relu-diff ops (DVE tensor_scalar / ACT Relu+bias),
    c-reduction via block-diagonal 0/1 selector matmuls (float32r -> full PE
    rate) into PSUM pre-seeded with -S/2 by an identity matmul, then one ACT
    op computes exp(-2*psum - S[b,i]) with accum_out row-sum -> o[b,i]
  - final FC done on device up to the (uncentered) logit; global mean
    correction + sigmoid on host (needs cross-core sum, trivially small)
"""

import numpy as np
from contextlib import ExitStack

N = 512
NB = 32
NCC = 16
CH = 20
H = 128
NCORES = 8
NPC = N // NCORES  # rows per core

# (i, k) relu-tile ops assigned to ACT instead of DVE (engine load balance)
ACT_TAKES = lambda i, k: k == 3 and (i % 4) != 3

_CACHE = {}


def _selector():
    # sel[:, 32k:32k+32] is the lhsT for MT tile k: maps the 128 bc-pairs
    # (bc = 128k + l, b = bc//16) onto the 32 output partitions b.
    sel = np.zeros((H, 4 * NB), dtype=np.float32)
    for k in range(4):
        for l in range(H):
            b = (128 * k + l) // NCC
            sel[l, 32 * k + b] = 1.0
    return sel


def _build():
    from concourse import bacc, bass, tile, mybir

    f32 = mybir.dt.float32
    f32r = mybir.dt.float32r
    AF = mybir.ActivationFunctionType
    OP = mybir.AluOpType

    nc = bacc.Bacc()

    # ---- I/O ----
    xT_d = nc.declare_dram_parameter("xT", [CH, N], f32, isOutput=False)
    xTo_d = nc.declare_dram_parameter("xTown", [CH, NPC], f32, isOutput=False)
    W1_d = nc.declare_dram_parameter("W1", [CH, H], f32, isOutput=False)
    W2_d = nc.declare_dram_parameter("W2", [H, CH], f32, isOutput=False)
    T_d = nc.declare_dram_parameter("T", [CH, N], f32, isOutput=False)
    Tsum_d = nc.declare_dram_parameter("Tsum", [CH, NB], f32, isOutput=False)
    b1_d = nc.declare_dram_parameter("b1", [H, 1], f32, isOutput=False)
    b2_d = nc.declare_dram_parameter("b2", [CH, 1], f32, isOutput=False)
    W3h_d = nc.declare_dram_parameter("W3h", [CH, 1], f32, isOutput=False)
    W3o_d = nc.declare_dram_parameter("W3o", [NB, 1], f32, isOutput=False)
    sel_d = nc.declare_dram_parameter("sel", [H, 4 * NB], f32r, isOutput=False)
    I32_d = nc.declare_dram_parameter("I32", [NB, NB], f32r, isOutput=False)

    logit_d = nc.declare_dram_parameter("logit", [NPC, 1], f32, isOutput=True)
    oall_d = nc.declare_dram_parameter("o_all", [NB, NPC], f32, isOutput=True)

    with tile.TileContext(nc) as tc, ExitStack() as ctx:
        const = ctx.enter_context(tc.tile_pool(name="const", bufs=1))
        work = ctx.enter_context(tc.tile_pool(name="work", bufs=1))
        relup = ctx.enter_context(tc.tile_pool(name="relup", bufs=8))
        expp = ctx.enter_context(tc.tile_pool(name="expp", bufs=4))
        psA = ctx.enter_context(
            tc.tile_pool(name="psA", bufs=2, space=bass.MemorySpace.PSUM)
        )
        psD = ctx.enter_context(
            tc.tile_pool(name="psD", bufs=4, space=bass.MemorySpace.PSUM)
        )
        psL = ctx.enter_context(
            tc.tile_pool(name="psL", bufs=1, space=bass.MemorySpace.PSUM)
        )

        # ---- load constants ----
        def load(name, dram, shape, dt=f32):
            t = const.tile(shape, dt, tag=name)
            nc.sync.dma_start(t[:], dram[:])
            return t

        xT = load("xT", xT_d, [CH, N])
        xTo = load("xTo", xTo_d, [CH, NPC])
        W1 = load("W1", W1_d, [CH, H])
        W2 = load("W2", W2_d, [H, CH])
        Tm = load("T", T_d, [CH, N])
        Tsum = load("Tsum", Tsum_d, [CH, NB])
        b1 = load("b1", b1_d, [H, 1])
        b2 = load("b2", b2_d, [CH, 1])
        W3h = load("W3h", W3h_d, [CH, 1])
        W3o = load("W3o", W3o_d, [NB, 1])
        sel = load("sel", sel_d, [H, 4 * NB], dt=f32r)
        I32 = load("I32", I32_d, [NB, NB], dt=f32r)

        # ---- phase A ----
        def mm(lhsT, rhs, mshape):
            ps = psA.tile(mshape, f32, tag="psa")
            nc.tensor.matmul(ps[:], lhsT, rhs, start=True, stop=True)
            return ps

        def mm_relu(lhsT, rhs, mshape, bias, tag):
            ps = mm(lhsT, rhs, mshape)
            out = work.tile(mshape, f32, tag=tag)
            nc.scalar.activation(out[:], ps[:], AF.Relu, bias=bias[:], scale=1.0)
            return out

        def mm_copy(lhsT, rhs, mshape, tag, dt=f32, scale=1.0):
            ps = mm(lhsT, rhs, mshape)
            out = work.tile(mshape, dt, tag=tag)
            nc.vector.tensor_scalar(out[:], ps[:], scale, None, OP.mult)
            return out

        # full-N side
        h1T = mm_relu(W1[:], xT[:], [H, N], b1, "h1T")
        h2T = mm_relu(W2[:], h1T[:], [CH, N], b2, "h2T")
        MT = [
            mm_copy(Tm[:, 128 * k : 128 * (k + 1)], h2T[:], [H, N], f"mt{k}")
            for k in range(4)
        ]
        # nhS2 = -S/2 (f32r) where S[b, j] = sum_c M[j, b, c]
        nhS2 = mm_copy(Tsum[:], h2T[:], [NB, N], "nhS2", dt=f32r, scale=-0.5)

        # own-rows side
        h1o = mm_relu(W1[:], xTo[:], [H, NPC], b1, "h1o")
        h2o = mm_relu(W2[:], h1o[:], [CH, NPC], b2, "h2o")
        Mo = [
            mm_copy(Tm[:, 128 * k : 128 * (k + 1)], h2o[:], [H, NPC], f"mo{k}")
            for k in range(4)
        ]
        negMo = []
        for k in range(4):
            t = work.tile([H, NPC], f32, tag=f"nmo{k}")
            nc.vector.tensor_scalar(t[:], Mo[k][:], -1.0, None, OP.mult)
            negMo.append(t)
        # negSo = -S[:, own rows]  (exp bias)
        negSo = mm_copy(Tsum[:], h2o[:], [NB, NPC], "negSo", scale=-1.0)

        o_all = work.tile([NB, NPC], f32, tag="o_all")

        # ---- hot loop ----
        for i in range(NPC):
            dps = psD.tile([NB, N], f32, tag="dps")
            # seed psum with -S/2
            nc.tensor.matmul(dps[:], I32[:], nhS2[:], start=True, stop=False)
            for k in range(4):
                r = relup.tile([H, N], f32r, tag="r")
                if ACT_TAKES(i, k):
                    nc.scalar.activation(
                        r[:], MT[k][:], AF.Relu, bias=negMo[k][:, i : i + 1], scale=1.0
                    )
                else:
                    nc.vector.tensor_scalar(
                        r[:],
                        MT[k][:],
                        Mo[k][:, i : i + 1],
                        0.0,
                        OP.subtract,
                        OP.max,
                    )
                nc.tensor.matmul(
                    dps[:],
                    sel[:, 32 * k : 32 * (k + 1)],
                    r[:],
                    start=False,
                    stop=(k == 3),
                )
            e = expp.tile([NB, N], f32, tag="e")
            nc.scalar.activation(
                e[:],
                dps[:],
                AF.Exp,
                scale=-2.0,
                bias=negSo[:, i : i + 1],
                accum_out=o_all[:, i : i + 1],
            )

        # ---- logits: h2o^T @ W3h + o_all^T @ W3o ----
        lps = psL.tile([NPC, 1], f32, tag="lps")
        nc.tensor.matmul(lps[:], h2o[:], W3h[:], start=True, stop=False)
        nc.tensor.matmul(
            lps[:], o_all[:], W3o[:], start=False, stop=True, skip_group_check=True
        )
        logit = work.tile([NPC, 1], f32, tag="logit")
        nc.vector.tensor_copy(logit[:], lps[:])

        nc.sync.dma_start(logit_d[:], logit[:])
        nc.sync.dma_start(oall_d[:], o_all[:])

    nc.finalize()
    return nc


def _get_nc():
    if "nc" not in _CACHE:
        _CACHE["nc"] = _build()
    return _CACHE["nc"]


def _run(inputs, trace=False):
    from concourse.bass_utils import run_bass_kernel_spmd

    x = np.ascontiguousarray(np.asarray(inputs["x"], dtype=np.float32))
    W1 = np.ascontiguousarray(np.asarray(inputs["W1"], dtype=np.float32))
    b1 = np.asarray(inputs["b1"], dtype=np.float32)
    W2 = np.ascontiguousarray(np.asarray(inputs["W2"], dtype=np.float32))
    b2 = np.asarray(inputs["b2"], dtype=np.float32)
    T = np.ascontiguousarray(np.asarray(inputs["T"], dtype=np.float32))
    W3 = np.asarray(inputs["W3"], dtype=np.float32)
    b3 = np.asarray(inputs["b3"], dtype=np.float32)

    xT = np.ascontiguousarray(x.T)  # (20, 512)
    common = {
        "xT": xT,
        "W1": W1,
        "W2": W2,
        "T": T,
        "Tsum": np.ascontiguousarray(T.reshape(CH, NB, NCC).sum(axis=2)),
        "b1": np.ascontiguousarray(b1.reshape(H, 1)),
        "b2": np.ascontiguousarray(b2.reshape(CH, 1)),
        "W3h": np.ascontiguousarray(W3[:CH].reshape(CH, 1)),
        "W3o": np.ascontiguousarray(W3[CH:].reshape(NB, 1)),
        "sel": _selector(),
        "I32": np.eye(NB, dtype=np.float32),
    }
    in_maps = [
        {**common, "xTown": np.ascontiguousarray(xT[:, NPC * k : NPC * (k + 1)])}
        for k in range(NCORES)
    ]

    nc = _get_nc()
    res = run_bass_kernel_spmd(nc, in_maps, list(range(NCORES)), trace=trace)

    o = np.zeros((N, NB), dtype=np.float64)
    logits = np.zeros((N,), dtype=np.float64)
    for k in range(NCORES):
        out = res.results[k]
        o[NPC * k : NPC * (k + 1), :] = out["o_all"].T
        logits[NPC * k : NPC * (k + 1)] = out["logit"][:, 0]

    mean_o = o.mean()
    w3o_sum = float(W3[CH:, 0].sum())
    logits = logits + float(b3[0]) - mean_o * w3o_sum
    out = 1.0 / (1.0 + np.exp(-logits))
    return out.reshape(N, 1).astype(np.float32), res


def kernel(**inputs) -> np.ndarray:
    out, _ = _run(inputs, trace=False)
    return out


def _install_ntff_hook():
    """The agent image's antenv lacks axon_hooks; bridge the boot shim's
    ctypes NTFF profiler into a stub module so trace=True works."""
    import sys
    import types

    if "antenv.axon_hooks" in sys.modules:
        return
    try:
        from trn_agent_boot.trn_boot import _ntff_profile_via_ctypes

        hook = _ntff_profile_via_ctypes("/opt/axon/libaxon_pjrt.so")
    except Exception:
        hook = None
    mod = types.ModuleType("antenv.axon_hooks")
    store = {"h": hook}
    mod.get_axon_ntff_profile_hook = lambda: store["h"]
    mod.set_axon_ntff_profile_hook = lambda h: store.update(h=h)
    sys.modules["antenv.axon_hooks"] = mod


def run_traced(inputs):
    """Returns (output, BassKernelResults) with profiling enabled."""
    _install_ntff_hook()
    return _run(inputs, trace=True)


# revision 11
# speedup vs baseline: 1.4654x; 1.4654x over previous
"""Trainium2 Bass kernel for nn_Discriminator_712964571446.

Reference computation (N=512, n_B=32, n_C=16, CH=20):
    h  = relu(relu(x@W1+b1)@W2+b2)                 # (N, 20)
    M  = (h @ T).reshape(N, 32, 16)
    d[i,b,j] = sum_c |M[i,b,c] - M[j,b,c]|
    o  = sum_j exp(-d) ; o -= mean(o)
    out = sigmoid(concat([h, o]) @ W3 + b3)        # (N, 1)

Strategy (8 cores, data parallel over N, 64 rows per core):
  - tiny FC layers + full M^T recomputed on every core (cheaper than collectives)
  - M^T stored as 4 fp16 SBUF tiles of (128 bc-pairs, 512 j)
  - |x| = 2 relu(x) - x, so with S~[b,j] = sum_c M~[j,b,c] (fp16-rounded M):
        d[b,j] = 2 * sum_c relu(M~_j - M_i) - S~[b,j] + S[b,i]
    per own row i: 4 fused relu-diff ops (DVE tensor_scalar sub+max at 4x
    fp16 mode / ACT Relu+bias), c-reduction via block-diagonal 0/1 fp16
    selector matmuls (1 cyc/row) into a PSUM quarter-bank
  - rows processed in groups of 4 sharing one full (128, 512) PSUM bank:
    one K=32 f32r seed matmul writes -S~/2 into all four quarters, the 16
    selector matmuls accumulate into their quarter (tile_position col
    groups), then a single ACT op computes exp(-2*psum - S[b,i]) over all
    128 partitions with accum_out row-sums -> o for 4 rows at once
  - logits-h on device; o @ W3o, global mean correction + sigmoid on host
    (cross-core mean is needed anyway, and the math is O(N) tiny)
"""

import numpy as np
from contextlib import ExitStack

N = 512
NB = 32
NCC = 16
CH = 20
H = 128
NCORES = 8
NPC = N // NCORES  # rows per core
NG = NPC // 4  # 4-row groups per core

# (i, k) relu-tile ops assigned to ACT instead of DVE (engine load balance)
ACT_TAKES = lambda i, k: k == 3 and (i % 4) != 3

_CACHE = {}


def _selector():
    # sel[:, 32k:32k+32] is the lhsT for MT tile k: maps the 128 bc-pairs
    # (bc = 128k + l, b = bc//16) onto the 32 output partitions b.
    sel = np.zeros((H, 4 * NB), dtype=np.float32)
    for k in range(4):
        for l in range(H):
            b = (128 * k + l) // NCC
            sel[l, 32 * k + b] = 1.0
    return sel


def _build():
    from concourse import bacc, bass, tile, mybir

    f32 = mybir.dt.float32
    f32r = mybir.dt.float32r
    f16 = mybir.dt.float16
    AF = mybir.ActivationFunctionType
    OP = mybir.AluOpType

    nc = bacc.Bacc()

    # ---- I/O ----
    xT_d = nc.declare_dram_parameter("xT", [CH, N], f32, isOutput=False)
    xTo_d = nc.declare_dram_parameter("xTown", [CH, NPC], f32, isOutput=False)
    W1_d = nc.declare_dram_parameter("W1", [CH, H], f32, isOutput=False)
    W2_d = nc.declare_dram_parameter("W2", [H, CH], f32, isOutput=False)
    T_d = nc.declare_dram_parameter("T", [CH, N], f32, isOutput=False)
    Tsum_d = nc.declare_dram_parameter("Tsum", [CH, NB], f32, isOutput=False)
    b1_d = nc.declare_dram_parameter("b1", [H, 1], f32, isOutput=False)
    b2_d = nc.declare_dram_parameter("b2", [CH, 1], f32, isOutput=False)
    W3h_d = nc.declare_dram_parameter("W3h", [CH, 1], f32, isOutput=False)
    sel_d = nc.declare_dram_parameter("sel", [H, 4 * NB], f16, isOutput=False)
    I32x4_d = nc.declare_dram_parameter("I32x4", [NB, H], f32r, isOutput=False)

    logit_d = nc.declare_dram_parameter("logit", [NPC, 1], f32, isOutput=True)
    o4_d = nc.declare_dram_parameter("o4", [H, NG], f32, isOutput=True)

    with tile.TileContext(nc) as tc, ExitStack() as ctx:
        const = ctx.enter_context(tc.tile_pool(name="const", bufs=1))
        work = ctx.enter_context(tc.tile_pool(name="work", bufs=1))
        relup = ctx.enter_context(tc.tile_pool(name="relup", bufs=12))
        expp = ctx.enter_context(tc.tile_pool(name="expp", bufs=2))
        psA = ctx.enter_context(
            tc.tile_pool(name="psA", bufs=2, space=bass.MemorySpace.PSUM)
        )
        psD = ctx.enter_context(
            tc.tile_pool(name="psD", bufs=4, space=bass.MemorySpace.PSUM)
        )
        psL = ctx.enter_context(
            tc.tile_pool(name="psL", bufs=1, space=bass.MemorySpace.PSUM)
        )

        # ---- load constants ----
        def load(name, dram, shape, dt=f32):
            t = const.tile(shape, dt, tag=name)
            nc.sync.dma_start(t[:], dram[:])
            return t

        xT = load("xT", xT_d, [CH, N])
        xTo = load("xTo", xTo_d, [CH, NPC])
        W1 = load("W1", W1_d, [CH, H])
        W2 = load("W2", W2_d, [H, CH])
        Tm = load("T", T_d, [CH, N])
        Tsum = load("Tsum", Tsum_d, [CH, NB])
        b1 = load("b1", b1_d, [H, 1])
        b2 = load("b2", b2_d, [CH, 1])
        W3h = load("W3h", W3h_d, [CH, 1])
        sel = load("sel", sel_d, [H, 4 * NB], dt=f16)
        I32x4 = load("I32x4", I32x4_d, [NB, H], dt=f32r)

        # ---- phase A ----
        def mm(lhsT, rhs, mshape, start=True, stop=True, ps=None):
            if ps is None:
                ps = psA.tile(mshape, f32, tag="psa")
            nc.tensor.matmul(ps[:], lhsT, rhs, start=start, stop=stop)
            return ps

        def mm_relu(lhsT, rhs, mshape, bias, tag):
            ps = mm(lhsT, rhs, mshape)
            out = work.tile(mshape, f32, tag=tag)
            nc.scalar.activation(out[:], ps[:], AF.Relu, bias=bias[:], scale=1.0)
            return out

        def mm_copy(lhsT, rhs, mshape, tag, dt=f32, scale=1.0):
            ps = mm(lhsT, rhs, mshape)
            out = work.tile(mshape, dt, tag=tag)
            nc.vector.tensor_scalar(out[:], ps[:], scale, None, OP.mult)
            return out

        # full-N side (MT tiles in bf16)
        h1T = mm_relu(W1[:], xT[:], [H, N], b1, "h1T")
        h2T = mm_relu(W2[:], h1T[:], [CH, N], b2, "h2T")
        MT = [
            mm_copy(Tm[:, 128 * k : 128 * (k + 1)], h2T[:], [H, N], f"mt{k}", dt=f16)
            for k in range(4)
        ]
        # S~[b, j] = sum_c M~[j, b, c] from the bf16 MT tiles (rounding must
        # match the relu-diff inputs so the |x| identity stays exact)
        psS = psA.tile([NB, N], f32, tag="psa")
        for k in range(4):
            nc.tensor.matmul(
                psS[:],
                sel[:, 32 * k : 32 * (k + 1)],
                MT[k][:],
                start=(k == 0),
                stop=(k == 3),
            )
        nhS2 = work.tile([NB, N], f32r, tag="nhS2")  # -S~/2
        nc.vector.tensor_scalar(nhS2[:], psS[:], -0.5, None, OP.mult)

        # own-rows side (fp32 exact)
        h1o = mm_relu(W1[:], xTo[:], [H, NPC], b1, "h1o")
        h2o = mm_relu(W2[:], h1o[:], [CH, NPC], b2, "h2o")
        Mo = [
            mm_copy(Tm[:, 128 * k : 128 * (k + 1)], h2o[:], [H, NPC], f"mo{k}")
            for k in range(4)
        ]
        negMo = []
        for k in range(4):
            t = work.tile([H, NPC], f32, tag=f"nmo{k}")
            nc.vector.tensor_scalar(t[:], Mo[k][:], -1.0, None, OP.mult)
            negMo.append(t)
        # negSo[b, i] = -S[b, i] (fp32, exact — matches the fp32 Mo scalars)
        negSo = mm_copy(Tsum[:], h2o[:], [NB, NPC], "negSo", scale=-1.0)
        # negSo4[32a + b, g] = negSo[b, 4g + a]  (exp bias, 4-row stacked)
        negSo4 = work.tile([H, NG], f32, tag="negSo4")
        negSo_v = negSo[:].rearrange("b (g a) -> b g a", a=4)
        for a in range(4):
            nc.vector.tensor_copy(negSo4[32 * a : 32 * (a + 1), :], negSo_v[:, :, a])

        o4 = work.tile([H, NG], f32, tag="o4")

        # ---- hot loop: groups of 4 rows share one full PSUM bank ----
        for g in range(NG):
            dps = psD.tile([H, N], f32, tag="dps")
            # seed all 4 quarters with -S~/2 in one K=32 matmul
            nc.tensor.matmul(dps[:], I32x4[:], nhS2[:], start=True, stop=False)
            for a in range(4):
                i = 4 * g + a
                for k in range(4):
                    r = relup.tile([H, N], f16, tag="r")
                    if ACT_TAKES(i, k):
                        nc.scalar.activation(
                            r[:],
                            MT[k][:],
                            AF.Relu,
                            bias=negMo[k][:, i : i + 1],
                            scale=1.0,
                        )
                    else:
                        nc.vector.tensor_scalar(
                            r[:],
                            MT[k][:],
                            Mo[k][:, i : i + 1],
                            0.0,
                            OP.subtract,
                            OP.max,
                        )
                    nc.tensor.matmul(
                        dps[32 * a : 32 * (a + 1), :],
                        sel[:, 32 * k : 32 * (k + 1)],
                        r[:],
                        start=False,
                        stop=(a == 3 and k == 3),
                        tile_position=(0, 32 * a),
                    )
            e = expp.tile([H, N], f32, tag="e")
            nc.scalar.activation(
                e[:],
                dps[:],
                AF.Exp,
                scale=-2.0,
                bias=negSo4[:, g : g + 1],
                accum_out=o4[:, g : g + 1],
            )

        # ---- logits-h: h2o^T @ W3h ----
        lps = psL.tile([NPC, 1], f32, tag="lps")
        nc.tensor.matmul(lps[:], h2o[:], W3h[:], start=True, stop=True)
        logit = work.tile([NPC, 1], f32, tag="logit")
        nc.vector.tensor_copy(logit[:], lps[:])

        nc.sync.dma_start(logit_d[:], logit[:])
        nc.sync.dma_start(o4_d[:], o4[:])

    nc.finalize()
    return nc


def _get_nc():
    if "nc" not in _CACHE:
        _CACHE["nc"] = _build()
    return _CACHE["nc"]


def _run(inputs, trace=False):
    from concourse.bass_utils import run_bass_kernel_spmd

    x = np.ascontiguousarray(np.asarray(inputs["x"], dtype=np.float32))
    W1 = np.ascontiguousarray(np.asarray(inputs["W1"], dtype=np.float32))
    b1 = np.asarray(inputs["b1"], dtype=np.float32)
    W2 = np.ascontiguousarray(np.asarray(inputs["W2"], dtype=np.float32))
    b2 = np.asarray(inputs["b2"], dtype=np.float32)
    T = np.ascontiguousarray(np.asarray(inputs["T"], dtype=np.float32))
    W3 = np.asarray(inputs["W3"], dtype=np.float32)
    b3 = np.asarray(inputs["b3"], dtype=np.float32)

    xT = np.ascontiguousarray(x.T)  # (20, 512)
    common = {
        "xT": xT,
        "W1": W1,
        "W2": W2,
        "T": T,
        "Tsum": np.ascontiguousarray(T.reshape(CH, NB, NCC).sum(axis=2)),
        "b1": np.ascontiguousarray(b1.reshape(H, 1)),
        "b2": np.ascontiguousarray(b2.reshape(CH, 1)),
        "W3h": np.ascontiguousarray(W3[:CH].reshape(CH, 1)),
        "sel": _selector().astype(np.float16),
        "I32x4": np.ascontiguousarray(
            np.concatenate([np.eye(NB, dtype=np.float32)] * 4, axis=1)
        ),
    }
    in_maps = [
        {**common, "xTown": np.ascontiguousarray(xT[:, NPC * k : NPC * (k + 1)])}
        for k in range(NCORES)
    ]

    nc = _get_nc()
    res = run_bass_kernel_spmd(nc, in_maps, list(range(NCORES)), trace=trace)

    o = np.zeros((N, NB), dtype=np.float64)
    logits = np.zeros((N,), dtype=np.float64)
    for c in range(NCORES):
        out = res.results[c]
        o4 = out["o4"]  # (128, 16): o4[32a + b, g] = o[4g + a (+64c), b]
        ob = o4.reshape(4, NB, NG).transpose(2, 0, 1).reshape(NPC, NB)
        o[NPC * c : NPC * (c + 1), :] = ob
        logits[NPC * c : NPC * (c + 1)] = out["logit"][:, 0]

    mean_o = o.mean()
    W3o = W3[CH:, 0].astype(np.float64)
    logits = logits + (o - mean_o) @ W3o + float(b3[0])
    out = 1.0 / (1.0 + np.exp(-logits))
    return out.reshape(N, 1).astype(np.float32), res


def kernel(**inputs) -> np.ndarray:
    out, _ = _run(inputs, trace=False)
    return out


def _install_ntff_hook():
    """The agent image's antenv lacks axon_hooks; bridge the boot shim's
    ctypes NTFF profiler into a stub module so trace=True works."""
    import sys
    import types

    if "antenv.axon_hooks" in sys.modules:
        return
    try:
        from trn_agent_boot.trn_boot import _ntff_profile_via_ctypes

        hook = _ntff_profile_via_ctypes("/opt/axon/libaxon_pjrt.so")
    except Exception:
        hook = None
    mod = types.ModuleType("antenv.axon_hooks")
    store = {"h": hook}
    mod.get_axon_ntff_profile_hook = lambda: store["h"]
    mod.set_axon_ntff_profile_hook = lambda h: store.update(h=h)
    sys.modules["antenv.axon_hooks"] = mod


def run_traced(inputs):
    """Returns (output, BassKernelResults) with profiling enabled."""
    _install_ntff_hook()
    return _run(inputs, trace=True)
